# revision 1
# baseline (speedup 1.0000x reference)
"""EuclideanFastAttention Trainium2 kernel (fp8 DoubleRow version).

Full inputs -> shard graphs across 8 NeuronCores (1 graph/core) -> per-core
Bass/Tile kernel (Euclidean RoPE + linear attention over Lebedev quadrature)
-> gather full output.

Design (per core, per antipodal pair term t with direction +-u):
  out += 2*w_t * [ (C.x) @ (C.k)^T V  +  (S.xs) @ (S.ks)^T V ]
with C/S = cos/sin(f_j u.r_n), xs the RoPE-swapped x. The d axis is stored
permuted (s,t,j)->(s,t-major,j) so the per-j cos/sin broadcast is a packed
4-dim AP. Matmuls run in fp8e4m3 DoubleRow (0.5 cyc/row); v is split
v = v_hi + v_lo (two-level fp8) to keep bf16-class accuracy on the value
path; q^T is produced by fp8 PE transposes into PSUM and moved to SBUF by
DMA (no compute-engine cost).

Self-contained: hardcodes the problem geometry (N=2048, B=8, P=1, S=4, F=64,
G=14, J=32) but derives everything it can from the input arrays at runtime.
"""
import sys

sys.path.insert(0, "/opt/trn_rl_repo")

import numpy as np

import concourse.bacc as bacc
import concourse.bass as bass
import concourse.mybir as mybir
import concourse.tile as tile
from concourse import masks
from concourse.bass_utils import run_bass_kernel_spmd

F32 = mybir.dt.float32
F32R = mybir.dt.float32r
BF16 = mybir.dt.bfloat16
E4 = mybir.dt.float8e4
ACTF = mybir.ActivationFunctionType
ALU = mybir.AluOpType
DR = mybir.MatmulPerfMode.DoubleRow

PI = float(np.pi)
TWO_PI = float(2.0 * np.pi)
INV_2PI = float(1.0 / (2.0 * np.pi))
MAGIC = float(1.5 * 2.0**23)  # fp32 round-to-nearest-int magic constant

N_CORES = 8
NT = 7        # antipodal pair terms
J = 32        # RoPE frequency pairs
D = 256       # p*s*f
M = 256       # nodes per graph
NK = 2        # 128-node chunks


def _ap(t_ap, off, dims):
    return bass.AP(tensor=t_ap.tensor, offset=t_ap.offset + off,
                   ap=[list(t_ap.ap[0])] + [list(d) for d in dims])


def _build_program():
    """SPMD per-core program. DRAM params:
    x   [128, 1024]  fp32: x[p, (c2, d256)], node = c*128+p  (original d order)
    aux [128, W]     fp32: col 0:2 mask (per chunk); rows 0:3 of later cols:
                     posT [3, 256], uT [3, 7] (pair reps), row 0: freq [32],
                     w_pair [7] (w_i + w_j per pair, from grid_w input values
                     combined on device -- here passed as the 14 grid_w values
                     plus pair index bookkeeping done host-side via column
                     order: wA [7] then wB [7])
    out [128, 1024]  fp32
    """
    c_mask = 0
    c_post = NK
    c_ut = c_post + M
    c_frq = c_ut + NT
    c_wa = c_frq + J
    c_wb = c_wa + NT
    W = c_wb + NT

    nc = bacc.Bacc()
    X = nc.declare_dram_parameter("x", [128, NK * D], F32, isOutput=False)
    AUX = nc.declare_dram_parameter("aux", [128, W], F32, isOutput=False)
    OUT = nc.declare_dram_parameter("out", [128, NK * D], F32, isOutput=True)

    with tile.TileContext(nc) as tc:
        with (
            tc.tile_pool(name="const", bufs=1) as cp,
            tc.tile_pool(name="kbf", bufs=4) as kbfp,
            tc.tile_pool(name="qt", bufs=4) as qtp,
            tc.tile_pool(name="kv8", bufs=4) as kv8p,
            tc.tile_pool(name="ang", bufs=2) as angp,
            tc.tile_pool(name="scb", bufs=3) as scp,
            tc.tile_pool(name="qtps", bufs=3, space="PSUM") as qtps,
            tc.tile_pool(name="kvps", bufs=2, space="PSUM") as kvps,
            tc.tile_pool(name="outps", bufs=1, space="PSUM") as outps,
        ):
            # ---------------- loads ----------------
            x_sb = cp.tile([128, NK * D], F32)
            aux_sb = cp.tile([128, W], F32)
            nc.sync.dma_start(out=aux_sb[0:3, c_post:W], in_=AUX[0:3, c_post:W])
            nc.sync.dma_start(out=aux_sb[:, 0:NK], in_=AUX[:, 0:NK])
            # x on the Pool DGE queue so it doesn't serialize behind aux
            nc.gpsimd.dma_start(out=x_sb, in_=X[:, :])

            zero_col = cp.tile([128, 1], F32)
            nc.vector.memset(zero_col, 0.0)

            # identities: f32 for the setup dp transpose, bf16 for k transposes
            identf = cp.tile([128, 128], F32)
            masks.make_identity(nc, identf[:])
            identb = cp.tile([128, 128], BF16)
            nc.vector.tensor_copy(identb, identf)

            # ---------------- angle pipeline (all terms, batched) -----------
            # dpT[p, (c2, t7)] = u_t . r_(c*128+p), computed directly:
            # lhsT = posT chunk [3, 128], rhs = uT [3, 7]
            out_ps_warm = outps.tile([128, 1024], BF16, tag="out",
                                     name="out_warm")
            set_ps = outps.tile([128, 512], F32, tag="out", name="set_ps")
            dpt_ps = set_ps[:, 0:2 * NT]
            for c in range(NK):
                nc.tensor.matmul(
                    dpt_ps[:, c * NT:(c + 1) * NT],
                    aux_sb[0:3, c_post + c * 128:c_post + (c + 1) * 128],
                    aux_sb[0:3, c_ut:c_ut + NT],
                    start=True, stop=True,
                )
            dpt = cp.tile([128, 2 * NT], F32)
            nc.vector.tensor_copy(dpt, dpt_ps)

            # freq / 2pi broadcast [128, J]
            frq_raw = cp.tile([128, J], F32)
            nc.gpsimd.partition_broadcast(frq_raw, aux_sb[0:1, c_frq:c_frq + J])
            frq_bc = cp.tile([128, J], F32)
            nc.vector.tensor_scalar_mul(frq_bc, frq_raw, INV_2PI)

            # w_term[p, t] = wA[t] + wB[t]  (the pair-trick factor 2 is already
            # absorbed: w*2*(A.KVA + B.KVB) = (wA+wB)(A.KVA + B.KVB))
            w_bc = cp.tile([128, 2 * NT], F32)
            nc.gpsimd.partition_broadcast(w_bc, aux_sb[0:1, c_wa:c_wa + 2 * NT])
            w_term = cp.tile([128, NT], F32)
            nc.gpsimd.tensor_add(w_term, w_bc[:, 0:NT], w_bc[:, NT:2 * NT])

            # Per-term angle chain (small ops, pipelined with the main loop):
            # w[p, (c, j)] = dpT[p, c, t] * freq[j]/2pi; kb cos/sin round
            # helpers (0.25 in separate ALU stages: fp32(MAGIC+0.25)==MAGIC);
            # d = w - kb; sc = Sin(2pi*d) -> bf16 [p, (sc2, c, j)].
            def emit_angle_pair(ts, fast=False):
                """Angle chain for 1-2 terms; one Sin op. fast=True runs the
                whole chain on DVE reading dots straight from PSUM (lowest
                serial latency, for the first terms); otherwise on Pool to
                keep DVE free in the steady-state loop."""
                n = len(ts)
                eng = nc.vector if fast else nc.gpsimd
                w_t = angp.tile([128, n * 2 * J], F32, tag="w", name="w_t")
                for i, t in enumerate(ts):
                    eng.tensor_mul(
                        w_t[:, i * 2 * J:(i + 1) * 2 * J].rearrange(
                            "p (c j) -> p c j", c=2),
                        _ap(dpt_ps if fast else dpt[:], t, [[NT, 2], [0, J]]),
                        _ap(frq_bc[:], 0, [[0, 2], [1, J]]),
                    )
                kb_t = angp.tile([128, n * 4 * J], F32, tag="kb", name="kb_t")
                tcos = angp.tile([128, n * 2 * J], F32, tag="tc", name="tcos")
                eng.tensor_scalar(tcos, w_t, 0.25, MAGIC, ALU.add, ALU.add)
                eng.tensor_scalar(
                    _ap(kb_t[:], 0, [[4 * J, n], [1, 2 * J]]),
                    tcos[:].rearrange("p (t cj) -> p t cj", t=n),
                    MAGIC, 0.25, ALU.subtract, ALU.subtract)
                eng.tensor_scalar(
                    _ap(kb_t[:], 2 * J, [[4 * J, n], [1, 2 * J]]),
                    w_t[:].rearrange("p (t cj) -> p t cj", t=n),
                    MAGIC, MAGIC, ALU.add, ALU.subtract)
                d_t = angp.tile([128, n * 4 * J], F32, tag="d", name="d_t")
                nc.vector.tensor_sub(
                    d_t[:].rearrange("p (t sc cj) -> p t sc cj", t=n, sc=2),
                    _ap(w_t[:], 0, [[2 * J, n], [0, 2], [1, 2 * J]]),
                    kb_t[:].rearrange("p (t sc cj) -> p t sc cj", t=n, sc=2),
                )
                sc_t = scp.tile([128, n * 4 * J], BF16, tag="sc", name="sc_t")
                nc.scalar.activation(sc_t, d_t, ACTF.Sin, bias=zero_col[:, 0:1],
                                     scale=TWO_PI)
                return {t: sc_t[:, i * 4 * J:(i + 1) * 4 * J]
                        for i, t in enumerate(ts)}

            # first two terms' angle chains on DVE, emitted before x-prep so
            # they run while the x DMA is still in flight
            # ---------------- x / v preparation ----------------
            # x_perm[p, (c, s, t, j)] bf16 from x[p, (c, s, j, t)]
            x_perm = cp.tile([128, NK * D], BF16)
            for t2 in range(2):
                nc.vector.tensor_copy(
                    _ap(x_perm[:], t2 * J, [[D, NK], [2 * J, 4], [1, J]]),
                    _ap(x_sb[:], t2, [[D, NK], [2 * J, 4], [2, J]]),
                )
            # xsw_perm: pair-swapped (-x2, x1) in permuted layout:
            # xsw[p, c, s, 0, j] = -x[p, c, s, j, t=1]; xsw[.., 1, j] = x[.., t=0]
            xsw_perm = cp.tile([128, NK * D], BF16)
            nc.vector.tensor_scalar_mul(
                _ap(xsw_perm[:], 0, [[D, NK], [2 * J, 4], [1, J]]),
                _ap(x_sb[:], 1, [[D, NK], [2 * J, 4], [2, J]]),
                -1.0,
            )
            nc.vector.tensor_scalar_mul(
                _ap(xsw_perm[:], J, [[D, NK], [2 * J, 4], [1, J]]),
                _ap(x_sb[:], 0, [[D, NK], [2 * J, 4], [2, J]]),
                1.0,
            )
            # v_bf = x_perm * mask (per chunk); bf16 kv keeps the value path
            # at bf16 accuracy (fp8 v fails the 2e-2 gate)
            v_bf = cp.tile([128, NK * D], BF16)
            for c in range(NK):
                nc.vector.tensor_scalar_mul(
                    v_bf[:, c * D:(c + 1) * D],
                    x_perm[:, c * D:(c + 1) * D],
                    aux_sb[:, c_mask + c:c_mask + c + 1],
                )

            # PE warm-up: dummy transposes into the (not yet started) out
            # bank keep the PE busy from ~1.3us so the p-state ramp reaches
            # full clock before the main loop; the first real out matmul's
            # start=True re-zeroes the region.
            for _ in range(24):
                nc.tensor.matmul(
                    out_ps_warm[:, 0:128], identb, identb, is_transpose=True,
                )

            # ---------------- main loop over pair terms ----------------
            # Software-pipelined emission: out(t-1) is emitted AFTER term t's
            # transposes/kv so the in-order PE never head-of-line blocks on
            # term t-1's escape copies.
            out_ps = outps.tile([128, NK * D], F32, tag="out", name="out_ps")
            n_out_mm = NT * 2 * NK
            out_count = [0]
            pending = {}

            def emit_muls(t, sc_ap):
                k_bf = kbfp.tile([128, 2 * NK * D], BF16, tag="kbf", name="k_bf")
                for part, src in ((0, x_perm), (1, xsw_perm)):
                    nc.vector.tensor_mul(
                        k_bf[:, part * 512:part * 512 + 512].rearrange(
                            "p (c st j) -> p c st j", c=2, st=8),
                        src[:].rearrange("p (c st j) -> p c st j", c=2, st=8),
                        _ap(sc_ap, part * 2 * J,
                            [[J, 2], [0, 8], [1, J]]),
                    )
                return k_bf

            def emit_pe_and_escapes(t, k_bf):
                # bf16 transposes -> qT_ps[p, (part, dc, n)] (n = c*128+p)
                qt_ps = qtps.tile([128, 1024], BF16, tag="qt", name="qt_ps")
                for part in range(2):
                    for c in range(NK):
                        for dc in range(2):
                            nc.tensor.matmul(
                                qt_ps[:, part * 512 + dc * 256 + c * 128:
                                      part * 512 + dc * 256 + c * 128 + 128],
                                k_bf[:, part * 512 + c * 256 + dc * 128:
                                     part * 512 + c * 256 + dc * 128 + 128],
                                identb,
                                is_transpose=True,
                            )
                # kv bf16; kv_ps[p, (part, dc, e)] fp32 (2 banks)
                kv_ps = kvps.tile([128, 1024], F32, tag="kv", name="kv_ps")
                for part in range(2):
                    for dc in range(2):
                        for c in range(NK):
                            nc.tensor.matmul(
                                kv_ps[:, part * 512 + dc * 256:
                                      part * 512 + dc * 256 + 256],
                                k_bf[:, part * 512 + c * 256 + dc * 128:
                                     part * 512 + c * 256 + dc * 128 + 128],
                                v_bf[:, c * D:(c + 1) * D],
                                start=(dc == 0 and c == 0),
                                stop=(dc == 1 and c == NK - 1),
                            )
                # escapes: qT -> fp8 (ACT), kv -> fp8 with w scale (DVE/ACT)
                qt_sb = qtp.tile([128, 1024], E4, tag="qtsb", name="qt_sb")
                nc.scalar.activation(qt_sb, qt_ps, ACTF.Copy)
                kv8 = kv8p.tile([128, 1024], E4, tag="kv8", name="kv8")
                nc.vector.tensor_scalar_mul(kv8[:, 0:640], kv_ps[:, 0:640],
                                            w_term[:, t:t + 1])
                nc.scalar.activation(kv8[:, 640:1024], kv_ps[:, 640:1024],
                                     ACTF.Copy, scale=w_term[:, t:t + 1])
                return qt_sb, kv8

            def emit_out(qt_sb, kv8, last=False):
                if not last:
                    for part in range(2):
                        for c in range(NK):
                            out_count[0] += 1
                            nc.tensor.matmul(
                                out_ps[:, c * 256:c * 256 + 256],
                                _ap(qt_sb[:], part * 512 + c * 128,
                                    [[256, 2], [1, 128]]),
                                _ap(kv8[:], part * 512, [[256, 2], [1, 256]]),
                                start=(out_count[0] == 1),
                                stop=False,
                                perf_mode=DR,
                            )
                    return
                # last term: finish chunk 0 first (its stop closes the group
                # so the c0 tail copy starts while c1 is still on the PE)
                for c in range(NK):
                    for part in range(2):
                        nc.tensor.matmul(
                            out_ps[:, c * 256:c * 256 + 256],
                            _ap(qt_sb[:], part * 512 + c * 128,
                                [[256, 2], [1, 128]]),
                            _ap(kv8[:], part * 512, [[256, 2], [1, 256]]),
                            start=False,
                            stop=(c == 0 and part == 1),
                            skip_group_check=(c == 1),
                            perf_mode=DR,
                        )

            sc_tiles = {}
            sc_tiles.update(emit_angle_pair([0]))
            sc_tiles.update(emit_angle_pair([1]))
            for t in range(NT):
                k_bf = emit_muls(t, sc_tiles.pop(t))
                if t == 0:
                    sc_tiles.update(emit_angle_pair([2, 3, 4, 5, 6]))
                pending[t] = emit_pe_and_escapes(t, k_bf)
                if t - 1 in pending:
                    emit_out(*pending.pop(t - 1))
            emit_out(*pending.pop(NT - 1), last=True)

            # ---------------- tail: mask + un-permute + store ----------------
            # out_sb[p, (s, j, t2)] = mask * out_ps[p, (s, t2, j)]; chunk 0 on
            # ACT and chunk 1 on DVE run concurrently (separate tiles), then
            # two output DMAs on separate HWDGE queues.
            o0 = cp.tile([128, D], F32)
            o1 = cp.tile([128, D], F32)
            nc.scalar.activation(
                _ap(o0[:], 0, [[2 * J, 4], [2, J], [1, 2]]),
                _ap(out_ps[:], 0, [[2 * J, 4], [1, J], [J, 2]]),
                ACTF.Copy,
                scale=aux_sb[:, c_mask:c_mask + 1],
            )
            nc.vector.tensor_scalar_mul(
                _ap(o1[:], 0, [[2 * J, 4], [2, J], [1, 2]]),
                _ap(out_ps[:], D, [[2 * J, 4], [1, J], [J, 2]]),
                aux_sb[:, c_mask + 1:c_mask + 2],
            )
            nc.sync.dma_start(out=OUT[:, 0:D], in_=o0)
            nc.scalar.dma_start(out=OUT[:, D:2 * D], in_=o1)

    nc.finalize()
    return nc


_PROGRAM_CACHE = {}


def _get_program():
    if "p" not in _PROGRAM_CACHE:
        _PROGRAM_CACHE["p"] = _build_program()
    return _PROGRAM_CACHE["p"]


def _find_pairs(grid_u, grid_w):
    """Antipodal pairs with equal weights; assert full pairing."""
    G = grid_u.shape[0]
    used = [False] * G
    pairs = []
    for i in range(G):
        if used[i]:
            continue
        partner = -1
        for j in range(i + 1, G):
            if used[j]:
                continue
            if (np.allclose(grid_u[j], -grid_u[i], rtol=1e-6, atol=1e-7)
                    and abs(float(grid_w[j]) - float(grid_w[i])) <= 1e-7):
                partner = j
                break
        used[i] = True
        assert partner >= 0, "unpaired grid direction"
        used[partner] = True
        pairs.append((i, partner))
    return pairs


def _prepare(inputs, positions, batch_segments, graph_mask, frequencies, grid_u,
             grid_w):
    n, p, s, f = inputs.shape
    d = p * s * f
    b = graph_mask.shape[0]
    G = grid_u.shape[0]
    Jn = frequencies.shape[0]
    assert (n, d, b, G, Jn) == (2048, 256, 8, 14, 32), (n, d, b, G, Jn)

    x = np.asarray(inputs, np.float32).reshape(n, d)
    pos = np.asarray(positions, np.float32)
    seg = np.asarray(batch_segments)
    gmask = np.asarray(graph_mask)
    gu = np.asarray(grid_u, np.float32)
    gw = np.asarray(grid_w, np.float32)

    idxs = [np.nonzero(seg == c)[0] for c in range(b)]
    assert max(len(ix) for ix in idxs) <= M

    pairs = _find_pairs(gu, gw)
    assert len(pairs) == NT
    reps = [i for i, _ in pairs]

    c_mask = 0
    c_post = NK
    c_ut = c_post + M
    c_frq = c_ut + NT
    c_wa = c_frq + J
    c_wb = c_wa + NT
    W = c_wb + NT

    in_maps = []
    for c in range(b):
        ix = idxs[c]
        pad = np.zeros(M, np.int64)
        pad[:len(ix)] = ix
        mask = np.zeros(M, np.float32)
        mask[:len(ix)] = gmask[seg[ix]].astype(np.float32)

        xs = x[pad]
        ps_ = pos[pad]
        x_prep = np.ascontiguousarray(
            xs.reshape(NK, 128, d).transpose(1, 0, 2).reshape(128, NK * d))
        aux = np.zeros((128, W), np.float32)
        aux[:, c_mask:c_mask + NK] = mask.reshape(NK, 128).T
        aux[0:3, c_post:c_post + M] = ps_.T
        aux[0:3, c_ut:c_ut + NT] = gu[reps].T
        aux[0, c_frq:c_frq + J] = np.asarray(frequencies, np.float32)
        aux[0, c_wa:c_wa + NT] = gw[[i for i, _ in pairs]]
        aux[0, c_wb:c_wb + NT] = gw[[j for _, j in pairs]]
        in_maps.append(dict(x=x_prep, aux=aux))

    meta = dict(n=n, p=p, s=s, f=f, d=d, b=b, idxs=idxs, pairs=pairs)
    return in_maps, meta


def _gather(results, meta, dtype):
    n, d = meta["n"], meta["d"]
    out = np.zeros((n, d), np.float32)
    for c, ix in enumerate(meta["idxs"]):
        o = results[c]["out"]
        o_nodes = o.reshape(128, NK, d).transpose(1, 0, 2).reshape(M, d)
        out[ix] = o_nodes[:len(ix)]
    return out.reshape(n, meta["p"], meta["s"], meta["f"]).astype(dtype)


def _run(inputs, positions, batch_segments, graph_mask, frequencies, grid_u,
         grid_w, trace=False):
    in_maps, meta = _prepare(inputs, positions, batch_segments, graph_mask,
                             frequencies, grid_u, grid_w)
    nc = _get_program()
    res = run_bass_kernel_spmd(
        nc, in_maps, core_ids=list(range(N_CORES)), trace=trace
    )
    out = _gather(res.results, meta, np.asarray(inputs).dtype)
    return out, res


def kernel(inputs, positions, batch_segments, graph_mask, frequencies, grid_u,
           grid_w):
    out, _ = _run(inputs, positions, batch_segments, graph_mask, frequencies,
                  grid_u, grid_w)
    return out



# revision 8
# speedup vs baseline: 1.0854x; 1.0854x over previous
"""EuclideanFastAttention Trainium2 kernel (Gram-matrix / A-form version).

Full inputs -> shard graphs across 8 NeuronCores (1 graph/core) -> per-core
Bass/Tile kernel (Euclidean RoPE + linear attention over Lebedev quadrature)
-> gather full output.

Math (per core = per graph, M=256 nodes, D=256 features, 7 antipodal pairs):
  out = sum_g w_g Q_g (K_g^T V)  ==  A @ V   with  A = sum_g w_g Q_g Q_g^T
(q and k are both the masked rotated x: masking k zeroes padded KV rows,
and masking q zeroes the same output rows the reference masks at the end,
so we fold the node mask into x once on the host). Per antipodal pair t the
cross terms cancel, and because a Gram matrix is invariant under the signed
pair-swap inside each (s,j) block ((S.xs)(S.xs)^T == (S.x)(S.x)^T), the
swapped operand xs disappears:
  A += (s_t c.x)(s_t c.x)^T + (s_t s.x)(s_t s.x)^T ,  s_t = sqrt(wA+wB).

Device pipeline per pair term t:
  k_bf = {cos_t, sin_t} * x         (DVE muls, bf16)
  qt   = k_bf^T                     (8 PE transposes -> PSUM bf16)
  qt8  = sqrt(w_t) * qt -> fp8      (ACT/DVE/Pool escape, w folded in scale)
  A   += qt8^T qt8                  (fp8 DoubleRow matmuls, K=256/instr)
A is accumulated in two PSUM banks (terms 0-4 and 5-6) so the first A@V
runs while the last terms are still in flight. out = A@V lands in PSUM f32,
is escaped to bf16 and DMA'd out; the host un-permutes and casts to f32.

Host prep folds: node mask into x, d-permutation (s,f)->(s,r,j), bf16
conversion, frequencies/(2*pi), sqrt of pair weights.

Self-contained: hardcodes the problem geometry (N=2048, B=8, P=1, S=4, F=64,
G=14, J=32) but derives everything it can from the input arrays at runtime.
"""
import sys

sys.path.insert(0, "/opt/trn_rl_repo")

import ml_dtypes
import numpy as np

import concourse.bacc as bacc
import concourse.bass as bass
import concourse.mybir as mybir
import concourse.tile as tile
from concourse import masks
from concourse.bass_utils import run_bass_kernel_spmd

F32 = mybir.dt.float32
BF16 = mybir.dt.bfloat16
E4 = mybir.dt.float8e4
ACTF = mybir.ActivationFunctionType
ALU = mybir.AluOpType
DR = mybir.MatmulPerfMode.DoubleRow

PI = float(np.pi)
TWO_PI = float(2.0 * np.pi)
MAGIC = float(1.5 * 2.0**23)  # fp32 round-to-nearest-int magic constant

N_CORES = 8
NT = 7        # antipodal pair terms
J = 32        # RoPE frequency pairs
D = 256       # p*s*f
M = 256       # nodes per graph
NK = 2        # 128-node chunks

# aux column map
C_POST = 0            # posT [3, 256]
C_UT = C_POST + M     # uT [3, 7]
C_FRQ = C_UT + NT     # freq/(2pi) [32] (row 0)
C_W = C_FRQ + J       # sqrt(wA+wB) [7] (row 0)
W_AUX = C_W + NT

NLATE = 5             # terms >= NLATE accumulate into A_late


def _ap(t_ap, off, dims):
    return bass.AP(tensor=t_ap.tensor, offset=t_ap.offset + off,
                   ap=[list(t_ap.ap[0])] + [list(d) for d in dims])


def _build_program():
    """SPMD per-core program. DRAM params:
    x   [128, 512] bf16: masked x, d-order (s,r,j), node = c*128+p
    aux [128, W_AUX] f32 (rows 0:3 used)
    out [128, 512] bf16: same layout as x
    """
    nc = bacc.Bacc()
    X = nc.declare_dram_parameter("x", [128, NK * D], BF16, isOutput=False)
    AUX = nc.declare_dram_parameter("aux", [128, W_AUX], F32, isOutput=False)
    OUT = nc.declare_dram_parameter("out", [128, NK * D], BF16, isOutput=True)

    with tile.TileContext(nc) as tc:
        with (
            tc.tile_pool(name="const", bufs=1) as cp,
            tc.tile_pool(name="kbf", bufs=4) as kbfp,
            tc.tile_pool(name="qt8", bufs=4) as qtp,
            tc.tile_pool(name="ang", bufs=2) as angp,
            tc.tile_pool(name="scb", bufs=2) as scp,
            tc.tile_pool(name="qtps", bufs=3, space="PSUM") as qtps,
            tc.tile_pool(name="accps", bufs=1, space="PSUM") as accps,
        ):
            # ---------------- loads ----------------
            aux_sb = cp.tile([128, W_AUX], F32)
            nc.sync.dma_start(out=aux_sb[0:3, :], in_=AUX[0:3, :])
            x_sb = cp.tile([128, NK * D], BF16)
            nc.scalar.dma_start(out=x_sb, in_=X[:, :])

            zero_col = cp.tile([128, 1], F32)
            nc.vector.memset(zero_col, 0.0)

            identf = cp.tile([128, 128], F32)
            masks.make_identity(nc, identf[:])
            identb = cp.tile([128, 128], BF16)
            nc.vector.tensor_copy(identb, identf)

            # ---------------- dpt / broadcasts ----------------
            # dpt[p, (c, t)] = u_t . r_(c*128+p)
            out_ps = accps.tile([128, 2 * D], F32, tag="out", name="out_ps")
            dpt_ps = out_ps[:, 0:2 * NT]
            for c in range(NK):
                nc.tensor.matmul(
                    dpt_ps[:, c * NT:(c + 1) * NT],
                    aux_sb[0:3, C_POST + c * 128:C_POST + (c + 1) * 128],
                    aux_sb[0:3, C_UT:C_UT + NT],
                    start=True, stop=True,
                )
            dpt = cp.tile([128, 2 * NT], F32)
            nc.vector.tensor_copy(dpt, dpt_ps)

            # freq/(2pi) broadcast [128, J] (host pre-divided)
            frq_bc = cp.tile([128, J], F32)
            nc.gpsimd.partition_broadcast(frq_bc, aux_sb[0:1, C_FRQ:C_FRQ + J])
            # sqrt(w) per term [128, NT]
            w_bc = cp.tile([128, NT], F32)
            nc.gpsimd.partition_broadcast(w_bc, aux_sb[0:1, C_W:C_W + NT])

            # ---------------- angle chains ----------------
            # w[p,(c,j)] = dpt[p,c,t] * f[j]/2pi; cos via +0.25 shift; range
            # reduce with MAGIC; sc = Sin(2pi d) -> bf16 [p,(sc2,c2,j32)].
            def emit_angle_pair(ts, fast=False):
                n = len(ts)
                eng = nc.vector if fast else nc.gpsimd
                w_t = angp.tile([128, n * 2 * J], F32, tag="w", name="w_t")
                for i, t in enumerate(ts):
                    eng.tensor_mul(
                        w_t[:, i * 2 * J:(i + 1) * 2 * J].rearrange(
                            "p (c j) -> p c j", c=2),
                        _ap(dpt_ps if fast else dpt[:], t, [[NT, 2], [0, J]]),
                        _ap(frq_bc[:], 0, [[0, 2], [1, J]]),
                    )
                kb_t = angp.tile([128, n * 4 * J], F32, tag="kb", name="kb_t")
                tcos = angp.tile([128, n * 2 * J], F32, tag="tc", name="tcos")
                eng.tensor_scalar(tcos, w_t, 0.25, MAGIC, ALU.add, ALU.add)
                eng.tensor_scalar(
                    _ap(kb_t[:], 0, [[4 * J, n], [1, 2 * J]]),
                    tcos[:].rearrange("p (t cj) -> p t cj", t=n),
                    MAGIC, 0.25, ALU.subtract, ALU.subtract)
                eng.tensor_scalar(
                    _ap(kb_t[:], 2 * J, [[4 * J, n], [1, 2 * J]]),
                    w_t[:].rearrange("p (t cj) -> p t cj", t=n),
                    MAGIC, MAGIC, ALU.add, ALU.subtract)
                d_t = angp.tile([128, n * 4 * J], F32, tag="d", name="d_t")
                nc.vector.tensor_sub(
                    d_t[:].rearrange("p (t sc cj) -> p t sc cj", t=n, sc=2),
                    _ap(w_t[:], 0, [[2 * J, n], [0, 2], [1, 2 * J]]),
                    kb_t[:].rearrange("p (t sc cj) -> p t sc cj", t=n, sc=2),
                )
                sc_t = scp.tile([128, n * 4 * J], BF16, tag="sc", name="sc_t")
                nc.scalar.activation(sc_t, d_t, ACTF.Sin, bias=zero_col[:, 0:1],
                                     scale=TWO_PI)
                return {t: sc_t[:, i * 4 * J:(i + 1) * 4 * J]
                        for i, t in enumerate(ts)}

            # ---------------- per-term pipeline ----------------
            A_e = accps.tile([128, 2 * D], F32, tag="Ae", name="A_e")
            A_l = accps.tile([128, 2 * D], F32, tag="Al", name="A_l")

            def emit_muls(t, sc_ap):
                k_bf = kbfp.tile([128, 2 * NK * D], BF16, tag="kbf",
                                 name="k_bf")
                for part in range(2):
                    nc.vector.tensor_mul(
                        k_bf[:, part * 512:part * 512 + 512].rearrange(
                            "p (c st j) -> p c st j", c=2, st=8),
                        x_sb[:].rearrange("p (c st j) -> p c st j", c=2, st=8),
                        _ap(sc_ap, part * 2 * J, [[J, 2], [0, 8], [1, J]]),
                    )
                return k_bf

            def emit_qt(t, k_bf):
                # transposes: qt_ps[p=d%128, (part, dc, c, m)]
                qt_ps = qtps.tile([128, 1024], BF16, tag="qt", name="qt_ps")
                for part in range(2):
                    for c in range(NK):
                        for dc in range(2):
                            nc.tensor.matmul(
                                qt_ps[:, part * 512 + dc * 256 + c * 128:
                                      part * 512 + dc * 256 + c * 128 + 128],
                                k_bf[:, part * 512 + c * 256 + dc * 128:
                                     part * 512 + c * 256 + dc * 128 + 128],
                                identb,
                                is_transpose=True,
                            )
                # escape with sqrt(w) fold -> fp8
                qt_sb = qtp.tile([128, 1024], E4, tag="qtsb", name="qt_sb")
                w_col = w_bc[:, t:t + 1]
                nc.scalar.activation(qt_sb[:, 0:640], qt_ps[:, 0:640],
                                     ACTF.Copy, scale=w_col)
                nc.vector.tensor_scalar_mul(qt_sb[:, 640:1024],
                                            qt_ps[:, 640:1024], w_col)
                return qt_sb

            def emit_A(t, qt_sb):
                acc = A_e if t < NLATE else A_l
                first = t in (0, NLATE)
                last = t in (NLATE - 1, NT - 1)
                for part in range(2):
                    for mc in range(2):
                        nc.tensor.matmul(
                            acc[:, mc * 256:mc * 256 + 256],
                            _ap(qt_sb[:], part * 512 + mc * 128,
                                [[256, 2], [1, 128]]),
                            _ap(qt_sb[:], part * 512, [[256, 2], [1, 256]]),
                            start=(first and part == 0 and mc == 0),
                            stop=(last and part == 1 and mc == 1),
                            perf_mode=DR,
                        )

            def emit_AV(a_ps, a_sb, start, stop):
                # A_sb[p, (mc, m')] = A[mc*128+p, m']; symmetric A lets the
                # same tile serve as lhsT for both output chunks.
                nc.vector.tensor_copy(a_sb[:, 0:256], a_ps[:, 0:256])
                nc.scalar.activation(a_sb[:, 256:512], a_ps[:, 256:512],
                                     ACTF.Copy)
                for oc in range(2):
                    for kc in range(2):
                        nc.tensor.matmul(
                            out_ps[:, oc * 256:oc * 256 + 256],
                            a_sb[:, kc * 256 + oc * 128:
                                 kc * 256 + oc * 128 + 128],
                            x_sb[:, kc * 256:(kc + 1) * 256],
                            start=(start and oc == 0 and kc == 0),
                            stop=(stop and oc == 1 and kc == 1),
                        )

            sc_tiles = {}
            sc_tiles.update(emit_angle_pair([0], fast=True))
            sc_tiles.update(emit_angle_pair([1], fast=True))
            sc_tiles.update(emit_angle_pair([2, 3, 4], fast=False))

            A_sbE = cp.tile([128, 2 * D], BF16)
            A_sbL = cp.tile([128, 2 * D], BF16)

            pending = {}
            for t in range(NT):
                k_bf = emit_muls(t, sc_tiles.pop(t))
                if t == 0:
                    sc_tiles.update(emit_angle_pair([5, 6], fast=False))
                pending[t] = emit_qt(t, k_bf)
                if t - 1 in pending:
                    emit_A(t - 1, pending.pop(t - 1))
                if t == NLATE:
                    # A_early closed by t=4's matmuls -> escape + A@V now,
                    # overlapping terms 5-6.
                    emit_AV(A_e, A_sbE, start=True, stop=False)
            emit_A(NT - 1, pending.pop(NT - 1))
            emit_AV(A_l, A_sbL, start=False, stop=True)

            # ---------------- tail: escape + store ----------------
            out_sb = cp.tile([128, NK * D], BF16)
            nc.scalar.activation(out_sb[:, 0:256], out_ps[:, 0:256], ACTF.Copy)
            nc.vector.tensor_copy(out_sb[:, 256:512], out_ps[:, 256:512])
            nc.sync.dma_start(out=OUT[:, 0:256], in_=out_sb[:, 0:256])
            nc.scalar.dma_start(out=OUT[:, 256:512], in_=out_sb[:, 256:512])

    nc.finalize()
    return nc


_PROGRAM_CACHE = {}


def _get_program():
    if "p" not in _PROGRAM_CACHE:
        _PROGRAM_CACHE["p"] = _build_program()
    return _PROGRAM_CACHE["p"]


def _find_pairs(grid_u, grid_w):
    """Antipodal pairs with equal weights; assert full pairing."""
    G = grid_u.shape[0]
    used = [False] * G
    pairs = []
    for i in range(G):
        if used[i]:
            continue
        partner = -1
        for j in range(i + 1, G):
            if used[j]:
                continue
            if (np.allclose(grid_u[j], -grid_u[i], rtol=1e-6, atol=1e-7)
                    and abs(float(grid_w[j]) - float(grid_w[i])) <= 1e-7):
                partner = j
                break
        used[i] = True
        assert partner >= 0, "unpaired grid direction"
        used[partner] = True
        pairs.append((i, partner))
    return pairs


def _prepare(inputs, positions, batch_segments, graph_mask, frequencies, grid_u,
             grid_w):
    n, p, s, f = inputs.shape
    d = p * s * f
    b = graph_mask.shape[0]
    G = grid_u.shape[0]
    Jn = frequencies.shape[0]
    assert (n, d, b, G, Jn) == (2048, 256, 8, 14, 32), (n, d, b, G, Jn)

    x = np.asarray(inputs, np.float32).reshape(n, d)
    pos = np.asarray(positions, np.float32)
    seg = np.asarray(batch_segments)
    gmask = np.asarray(graph_mask)
    gu = np.asarray(grid_u, np.float32)
    gw = np.asarray(grid_w, np.float32)

    idxs = [np.nonzero(seg == c)[0] for c in range(b)]
    assert max(len(ix) for ix in idxs) <= M

    pairs = _find_pairs(gu, gw)
    assert len(pairs) == NT
    reps = [i for i, _ in pairs]
    wpair = gw[[i for i, _ in pairs]] + gw[[j for _, j in pairs]]
    assert np.all(wpair > 0)

    in_maps = []
    for c in range(b):
        ix = idxs[c]
        pad = np.zeros(M, np.int64)
        pad[:len(ix)] = ix
        mask = np.zeros(M, np.float32)
        mask[:len(ix)] = gmask[seg[ix]].astype(np.float32)

        xm = x[pad] * mask[:, None]
        # d-permute (s, f=2j+r) -> (s, r, j)
        xp = xm.reshape(M, 4, J, 2).transpose(0, 1, 3, 2).reshape(M, d)
        x_prep = np.ascontiguousarray(
            xp.reshape(NK, 128, d).transpose(1, 0, 2).reshape(128, NK * d))
        aux = np.zeros((128, W_AUX), np.float32)
        aux[0:3, C_POST:C_POST + M] = pos[pad].T
        aux[0:3, C_UT:C_UT + NT] = gu[reps].T
        aux[0, C_FRQ:C_FRQ + J] = (np.asarray(frequencies, np.float32)
                                   / np.float32(TWO_PI))
        aux[0, C_W:C_W + NT] = np.sqrt(wpair.astype(np.float32))
        in_maps.append(dict(x=x_prep.astype(ml_dtypes.bfloat16), aux=aux))

    meta = dict(n=n, p=p, s=s, f=f, d=d, b=b, idxs=idxs, pairs=pairs)
    return in_maps, meta


def _gather(results, meta, dtype):
    n, d = meta["n"], meta["d"]
    out = np.zeros((n, d), np.float32)
    for c, ix in enumerate(meta["idxs"]):
        o = np.asarray(results[c]["out"]).astype(np.float32)
        o_nodes = o.reshape(128, NK, d).transpose(1, 0, 2).reshape(M, d)
        # un-permute (s, r, j) -> (s, f=2j+r)
        o_un = o_nodes.reshape(M, 4, 2, J).transpose(0, 1, 3, 2).reshape(M, d)
        out[ix] = o_un[:len(ix)]
    return out.reshape(n, meta["p"], meta["s"], meta["f"]).astype(dtype)


def _run(inputs, positions, batch_segments, graph_mask, frequencies, grid_u,
         grid_w, trace=False):
    in_maps, meta = _prepare(inputs, positions, batch_segments, graph_mask,
                             frequencies, grid_u, grid_w)
    nc = _get_program()
    res = run_bass_kernel_spmd(
        nc, in_maps, core_ids=list(range(N_CORES)), trace=trace
    )
    out = _gather(res.results, meta, np.asarray(inputs).dtype)
    return out, res


def kernel(inputs, positions, batch_segments, graph_mask, frequencies, grid_u,
           grid_w):
    out, _ = _run(inputs, positions, batch_segments, graph_mask, frequencies,
                  grid_u, grid_w)
    return out
